# revision 2
# baseline (speedup 1.0000x reference)
"""Trainium2 Bass kernel for nn_BaselineModel_55018531061929 (2-layer HSTU-style
dense transformer, B=2 L=2048 D=1024 H=8, SiLU attention).

Sharding: balanced-causal sequence-parallel. Each core owns 512 tokens of one
batch, but as the chunk pair (r, 7-r) of 256 tokens each, so causal attention
work is identical across cores: every core processes exactly 9 (query-chunk,
key-chunk) block pairs of 256x256. Which key chunks, and which of the two
query chunks each block uses, differ per core - driven by a host-provided
schedule table feeding dynamic (register-offset) DMA/matmul/PSUM addressing.
PSUM has_written semantics handle the per-core split of the AV accumulation
between the two query halves without any flags.

K and V are exchanged with ONE 8-core AllGather each per layer into a
"Shared" DRAM scratchpad (all 8 cores are on one chip, so the gather is
one local HBM write per core + barrier instead of a 4-ring broadcast).
Attention reads K/V blocks from the shared buffer with dynamic offsets.
The causal mask is applied to every computed block from input mask data.

All matmuls run with bf16 operands (weights cast on host, activations
written bf16) with fp32 PSUM accumulation; layernorm stats stay fp32. The
pre-projection layernorm affines are folded into the weights on the host.
Activations live feature-major in SBUF ([d partitions, tokens free]).
RoPE uses a host-side permutation of the Wq/Wk rows plus an on-device
half-swap matmul.
"""

import os
import time

import numpy as np

B, L, D, H, NL = 2, 2048, 1024, 8, 2
HD = D // H
EPS = 1e-8
NC = 8
T = 512            # tokens per core (two 256-token chunks)
CH = 256           # chunk size
DT = D // 128      # 8 d-tiles
NW = 9             # causal block-pairs per core per head
GROUP8 = [[0, 1, 2, 3, 4, 5, 6, 7]]

_CACHE = {}


def _sched_for_core(c):
    """Per-w (chunk m, query-half qh) schedule for core c.

    Diagonal-first: w=0 is query-chunk A's diagonal block and w=NW-1 is
    chunk B's diagonal block (the only two that need mask data); the rest
    are strictly-past (all-ones mask) blocks.
    """
    r = c % 4
    out = [(r, 0)]
    for w in range(1, r + 1):
        out.append((w - 1, 0))
    for w in range(r + 1, NW - 1):
        out.append((w - r - 1, 1))
    out.append((7 - r, 1))
    return out


# --------------------------------------------------------------------------
# device program
# --------------------------------------------------------------------------

def _build_program(sim=False, unroll=1):
    import concourse.bacc as bacc
    import concourse.bass as bass_mod
    import concourse.mybir as mybir
    import concourse.tile as tile
    from concourse.masks import make_identity

    # experiment knobs (timing experiments only; E_STATIC breaks correctness)
    E_LOCAL = os.environ.get("E_LOCAL", "") == "1"    # Local AG outs
    E_STATIC = os.environ.get("E_STATIC", "") == "1"  # static attn addressing
    E_NOAG = os.environ.get("E_NOAG", "") == "1"      # DMA fan-out, no coll.
    E_NOATTN = os.environ.get("E_NOATTN", "") == "1"  # skip attn inner loop
    E_SMM = os.environ.get("E_SMM", "") == "1"        # static matmul APs only
    E_AG1 = os.environ.get("E_AG1", "") == "1"        # one AG per layer
    # manual AG: each core publishes its K/V shard into the Shared buffer
    # with one dynamic-slot DMA; a tiny AllGather serves as the barrier.
    # BROKEN on this topology: the Shared scratchpad is only shared within
    # 2-core SEngine pairs, so plain DMA writes don't reach the other pairs
    # (probe: mb_manag.py). Keep off; the real AllGather handles routing.
    E_MANAG = os.environ.get("E_MANAG", "0") == "1"

    f32 = mybir.dt.float32
    f32r = mybir.dt.float32r
    bf16 = mybir.dt.bfloat16
    i32 = mybir.dt.int32
    AF = mybir.ActivationFunctionType
    ds = bass_mod.ds
    from concourse.tile_rust import add_dep_helper as _add_dep

    nc = bacc.Bacc("TRN2", target_bir_lowering=False, debug=False,
                   num_devices=1 if sim else NC)

    # ---- I/O ----
    x_in = nc.dram_tensor("x_fm", [D, T], f32r, kind="ExternalInput")
    mblk_in = nc.dram_tensor("mblk", [2, 2, 128, CH], bf16,
                             kind="ExternalInput")
    sched_in = nc.dram_tensor("sched", [1, NW * 4], i32, kind="ExternalInput")
    cos_in = nc.dram_tensor("cosf", [128, T], f32, kind="ExternalInput")
    sin_in = nc.dram_tensor("sinf", [128, T], f32, kind="ExternalInput")
    psw_in = nc.dram_tensor("pswap", [128, 128], f32r, kind="ExternalInput")
    w_in = nc.dram_tensor("wstack", [7 * NL, 8, 128, DT, 128], bf16,
                          kind="ExternalInput")
    ones_in = nc.dram_tensor("onesf", [128, 128], f32r, kind="ExternalInput")
    b_in = nc.dram_tensor("bstack", [7 * NL, 128, 8], f32, kind="ExternalInput")
    lng_in = nc.dram_tensor("lng", [2 * NL + 1, 128, DT], f32, kind="ExternalInput")
    lnb_in = nc.dram_tensor("lnb", [2 * NL + 1, 128, DT], f32, kind="ExternalInput")
    out_t = nc.dram_tensor("out_fm", [D, T], f32r, kind="ExternalOutput")

    W_Q, W_K, W_U, W_V, W_O, W_1, W_2 = range(7)
    INV_SQRT_HD = float(1.0 / np.sqrt(HD))

    with tile.TileContext(nc) as tc:
        with (
            tc.tile_pool(name="const", bufs=1) as constp,
            tc.tile_pool(name="acts", bufs=1) as acts,
            tc.tile_pool(name="wcol", bufs=8) as wcolp,
            tc.tile_pool(name="tmp", bufs=6) as tmpp,
            tc.tile_pool(name="small", bufs=4) as smallp,
            tc.tile_pool(name="krp", bufs=4) as krp,
            tc.tile_pool(name="vrp", bufs=2) as vrp,
            tc.tile_pool(name="kfp", bufs=6) as kfp,
            tc.tile_pool(name="wtsp", bufs=4) as wtsp,
            tc.tile_pool(name="bcp", bufs=1) as bcp,
            tc.tile_pool(name="psc", bufs=3, space="PSUM") as pscp,
            tc.tile_pool(name="shr", bufs=2, space="PSUM") as shrp,
            tc.tile_pool(name="dram", bufs=1, space="DRAM") as dramp,
        ):
            # ---- constants ----
            ones_sb = constp.tile([128, 128], f32r, name="ones_sb")
            nc.sync.dma_start(ones_sb[:], ones_in[:])
            ones_col = ones_sb[:, 0:1]
            ones_row = ones_sb[0:1, :]
            eps_col = constp.tile([128, 1], f32, name="eps_col")
            nc.vector.memset(eps_col[:], EPS)
            x_sb = constp.tile([128, DT, T], f32r, name="x_sb")
            x_in_t = x_in.ap().rearrange("(dt p) t -> p dt t", p=128)
            nc.sync.dma_start(x_sb[:], x_in_t[:])
            mask_sb = constp.tile([128, 2, 2, CH], bf16, name="mask_sb")
            nc.sync.dma_start(
                mask_sb[:], mblk_in.ap().rearrange("w k p q -> p w k q"))
            sched_sb = constp.tile([1, NW * 4], i32, name="sched_sb")
            nc.sync.dma_start(sched_sb[:], sched_in.ap())
            cos_sb = constp.tile([128, T], f32, name="cos_sb")
            nc.sync.dma_start(cos_sb[:], cos_in[:])
            sin_sb = constp.tile([128, T], f32, name="sin_sb")
            nc.sync.dma_start(sin_sb[:], sin_in[:])
            psw_sb = constp.tile([128, 128], f32r, name="psw_sb")
            nc.sync.dma_start(psw_sb[:], psw_in[:])
            bcol_sb = constp.tile([128, 7 * NL, 8], f32, name="bcol_sb")
            nc.sync.dma_start(bcol_sb[:], b_in.ap().rearrange("w p c -> p w c"))
            lng_sb = constp.tile([128, 2 * NL + 1, DT], f32, name="lng_sb")
            nc.sync.dma_start(lng_sb[:], lng_in.ap().rearrange("w p c -> p w c"))
            lnb_sb = constp.tile([128, 2 * NL + 1, DT], f32, name="lnb_sb")
            nc.sync.dma_start(lnb_sb[:], lnb_in.ap().rearrange("w p c -> p w c"))
            ident = constp.tile([128, 128], bf16, name="ident")
            make_identity(nc, ident)

            # ---- per-w dynamic schedule scalars (cidx, khalf, qoff, qh) ----
            sched_vals = []
            for w in range(NW):
                regs_c = nc.alloc_registers(f"sc_c{w}")
                nc.regs_load(regs_c, sched_sb[0:1, 4 * w:4 * w + 1])
                v_c = nc.snap(regs_c, donate=True, min_val=0, max_val=7)
                regs_s = nc.alloc_registers(f"sc_s{w}")
                nc.regs_load(regs_s, sched_sb[0:1, 4 * w + 1:4 * w + 2])
                v_s = nc.snap(regs_s, donate=True, min_val=0, max_val=1)
                regs_q = nc.alloc_registers(f"sc_q{w}")
                nc.regs_load(regs_q, sched_sb[0:1, 4 * w + 2:4 * w + 3])
                v_q = nc.snap(regs_q, donate=True, min_val=0, max_val=CH)
                regs_h = nc.alloc_registers(f"sc_h{w}")
                nc.regs_load(regs_h, sched_sb[0:1, 4 * w + 3:4 * w + 4])
                v_h = nc.snap(regs_h, donate=True, min_val=0, max_val=1)
                sched_vals.append((v_c, v_s, v_q, v_h))

            # ---- collective buffers ----
            # Combined K+V per layer, split into two otp-pair groups so the
            # second AllGather hides behind attention on the first group.
            # Layout: [otpg, half, otp2, j, kv, 128, 256] where kv=0 is K
            # ([128 hd, 256 keys]) and kv=1 is V ([128 keys, 2 kt x 128 hd]).
            # Shared AG outputs are single-writer: one tile per (rep, layer,
            # group).
            ag_kv_in = [dramp.tile([2, 2, 2, 2, 2, 128, CH], bf16,
                                   name=f"agkvi{l}")
                        for l in range(NL)]
            aspace = "Local" if (sim or E_LOCAL) else "Shared"
            if E_MANAG and not sim:
                ag_kv_cmb = [[dramp.tile([8, 2, 2, 2, 2, 2, 128, CH], bf16,
                                         name=f"agkvm{rep}_{l}",
                                         addr_space=aspace)
                              for l in range(NL)] for rep in range(unroll)]
                ag_kv_out = [[[ag_kv_cmb[rep][l][:, g] for g in range(2)]
                              for l in range(NL)] for rep in range(unroll)]
                bar_in = dramp.tile([1, CH], bf16, name="bar_in")
                bar_out = dramp.tile([8, 1, CH], bf16, name="bar_out")
            elif E_AG1:
                ag_kv_cmb = [[dramp.tile([8, 2, 2, 2, 2, 2, 128, CH], bf16,
                                         name=f"agkvc{rep}_{l}",
                                         addr_space=aspace)
                              for l in range(NL)] for rep in range(unroll)]
                ag_kv_out = [[[ag_kv_cmb[rep][l][:, g] for g in range(2)]
                              for l in range(NL)] for rep in range(unroll)]
            else:
                ag_kv_out = [[[dramp.tile([8, 2, 2, 2, 2, 128, CH], bf16,
                                          name=f"agkvo{rep}_{l}_{g}",
                                          addr_space=aspace)
                               for g in range(2)]
                              for l in range(NL)] for rep in range(unroll)]

            def all_gather(src, dst):
                if sim or E_NOAG:
                    for i_ in range(8):
                        nc.sync.dma_start(dst[i_], src[:])
                else:
                    nc.gpsimd.collective_compute(
                        "AllGather", mybir.AluOpType.bypass,
                        replica_groups=GROUP8,
                        ins=[src[:]], outs=[dst.opt()],
                    )

            def load_wpair(widx, otp):
                """One DMA fetching both ot tiles of a projection pair."""
                w = wcolp.tile([128, 2, DT, 128], bf16, name="wcp", tag="wct")
                nc.sync.dma_start(
                    w[:], w_in.ap()[widx, 2 * otp:2 * otp + 2].rearrange(
                        "o p dt c -> p o dt c"))
                return w

            def layernorm(idx, affine=False):
                """Normalize x_sb -> new 'bigA' tile, using ln row idx."""
                ps_sum = shrp.tile([1, T], f32, name="ps_sum", tag="shr",
                                   padded_shape=[128, T])
                ps_sq = shrp.tile([1, T], f32, name="ps_sq", tag="shr",
                                  padded_shape=[128, T])
                for dt in range(DT):
                    sqv = tmpp.tile([128, T], f32r, name="sqv", tag="tmp")
                    nc.scalar.square(sqv[:], x_sb[:, dt, :])
                    nc.tensor.matmul(ps_sum[:], ones_col[:], x_sb[:, dt, :],
                                     start=dt == 0, stop=dt == DT - 1)
                    nc.tensor.matmul(ps_sq[:], ones_col[:], sqv[:],
                                     start=dt == 0, stop=dt == DT - 1)
                s_mean = smallp.tile([1, T], f32, name="s_mean", tag="sm")
                nc.vector.tensor_scalar_mul(s_mean[:], ps_sum[:], 1.0 / D)
                s_var = smallp.tile([1, T], f32, name="s_var", tag="sm")
                nc.vector.tensor_scalar_mul(s_var[:], ps_sq[:], 1.0 / D)
                s_msq = smallp.tile([1, T], f32, name="s_msq", tag="sm")
                nc.vector.tensor_mul(s_msq[:], s_mean[:], s_mean[:])
                nc.vector.tensor_sub(s_var[:], s_var[:], s_msq[:])
                s_std = smallp.tile([1, T], f32, name="s_std", tag="sm")
                nc.scalar.activation(s_std[:], s_var[:], AF.Sqrt, bias=eps_col[:1])
                s_istd = smallp.tile([1, T], f32r, name="s_istd", tag="sm")
                with nc.allow_low_precision(reason="f32r is full-width fp32"):
                    nc.vector.reciprocal(s_istd[:], s_std[:])
                s_ms = smallp.tile([1, T], f32r, name="s_ms", tag="sm")
                nc.vector.tensor_mul(s_ms[:], s_mean[:], s_istd[:])
                bc = bcp.tile([128, 2, T], f32, name="bc", tag="bc")
                for k_, src_ in ((0, s_istd), (1, s_ms)):
                    b_ps = shrp.tile([128, T], f32, name="b_ps", tag="shr")
                    nc.tensor.matmul(b_ps[:], ones_row[:], src_[:],
                                     start=True, stop=True)
                    nc.scalar.activation(bc[:, k_, :], b_ps[:], AF.Identity)
                if affine:
                    h = acts.tile([128, DT, T], f32r, name="hf", tag="hf")
                else:
                    h = acts.tile([128, DT, T], bf16, name="h", tag="bigA")
                for dt in range(DT):
                    if affine:
                        t1 = tmpp.tile([128, T], f32, name="t1", tag="tmp")
                        nc.vector.tensor_mul(t1[:], x_sb[:, dt, :], bc[:, 0, :])
                        nc.vector.tensor_sub(t1[:], t1[:], bc[:, 1, :])
                        nc.scalar.activation(h[:, dt, :], t1[:], AF.Identity,
                                             bias=lnb_sb[:, idx, dt:dt + 1],
                                             scale=lng_sb[:, idx, dt:dt + 1])
                    else:
                        t1 = tmpp.tile([128, T], f32, name="t1", tag="tmp")
                        nc.vector.tensor_mul(t1[:], x_sb[:, dt, :], bc[:, 0, :])
                        nc.vector.tensor_sub(h[:, dt, :], t1[:], bc[:, 1, :])
                return h

            def proj_pair_psum(widx, otp, rhs_tile):
                """[128, 2, T] psum: halves = ot 2*otp, 2*otp+1 accumulation."""
                w = load_wpair(widx, otp)
                ps = pscp.tile([128, 2, T], f32, name="ps_p", tag="psc")
                for dt in range(DT):
                    nc.tensor.matmul(ps[:, 0, :], w[:, 0, dt, :],
                                     rhs_tile[:, dt, :],
                                     start=dt == 0, stop=dt == DT - 1)
                    nc.tensor.matmul(ps[:, 1, :], w[:, 1, dt, :],
                                     rhs_tile[:, dt, :],
                                     start=dt == 0, stop=dt == DT - 1)
                return ps

            def rope_into(dst_ap, src_tile):
                """dst = src*cosf + (pswap@src)*sinf (single rounding)."""
                psw = shrp.tile([128, T], f32, name="psw_ps", tag="shr")
                nc.tensor.matmul(psw[:], psw_sb[:], src_tile[:],
                                 start=True, stop=True)
                t3 = tmpp.tile([128, T], f32, name="rt3", tag="tmp")
                nc.vector.tensor_mul(t3[:], src_tile[:], cos_sb[:])
                t2 = tmpp.tile([128, T], f32, name="rt2", tag="tmp")
                nc.vector.tensor_mul(t2[:], psw[:], sin_sb[:])
                nc.vector.tensor_add(dst_ap, t3[:], t2[:])

            for rep in range(unroll):
                if rep > 0:
                    nc.sync.dma_start(x_sb[:], x_in_t[:])
                for layer in range(NL):
                    wofs = 7 * layer
                    h = layernorm(2 * layer)

                    # ---- K projection + rope -> shared AG ----
                    for otp in range(H // 2):
                        ps = proj_pair_psum(wofs + W_K, otp, h)
                        for j in range(2):
                            ot = 2 * otp + j
                            ktmp = tmpp.tile([128, T], f32r, name="ktmp",
                                             tag="tmp")
                            nc.scalar.activation(
                                ktmp[:], ps[:, j, :], AF.Identity,
                                bias=bcol_sb[:, wofs + W_K, ot:ot + 1])
                            kr = krp.tile([128, T], bf16, name="kr", tag="kr")
                            rope_into(kr[:], ktmp)
                            nc.sync.dma_start(
                                ag_kv_in[layer][otp // 2, :, otp % 2, j, 0]
                                .rearrange("h p q -> p h q"),
                                kr[:].rearrange("p (h q) -> p h q", h=2))

                    # ---- V projection + producer-side transpose -> AG ----
                    for otp in range(H // 2):
                        ps = proj_pair_psum(wofs + W_V, otp, h)
                        vr = vrp.tile([128, 2, 2, 2, 128], bf16, name="vr",
                                      tag="vr")
                        for j in range(2):
                            ot = 2 * otp + j
                            vtmp = tmpp.tile([128, T], bf16, name="vtmp",
                                             tag="tmpb")
                            nc.scalar.activation(
                                vtmp[:], ps[:, j, :], AF.Identity,
                                bias=bcol_sb[:, wofs + W_V, ot:ot + 1])
                            for jj in range(4):
                                pst = shrp.tile([128, 128], bf16, name="pst",
                                                tag="shr")
                                nc.tensor.transpose(
                                    pst[:], vtmp[:, jj * 128:(jj + 1) * 128],
                                    ident[:])
                                nc.vector.tensor_copy(
                                    vr[:, j, jj // 2, jj % 2, :], pst[:])
                        for j in range(2):
                            for hf in range(2):
                                nc.sync.dma_start(
                                    ag_kv_in[layer][otp // 2, hf, otp % 2,
                                                    j, 1].rearrange(
                                        "p (k c) -> p k c", k=2),
                                    vr[:, j, hf])
                        if E_MANAG and not sim:
                            if otp == H // 2 - 1:
                                pid8 = nc.partition_id()
                                cp = nc.sync.dma_start(
                                    ag_kv_cmb[rep][layer][bass_mod.ds(pid8, 1)],
                                    ag_kv_in[layer][:].rearrange(
                                        "(a g) h o j v p q -> a g h o j v p q",
                                        a=1))
                                kv_bar = nc.gpsimd.collective_compute(
                                    "AllGather", mybir.AluOpType.bypass,
                                    replica_groups=GROUP8,
                                    ins=[bar_in[:]], outs=[bar_out.opt()])
                                _add_dep(kv_bar.ins, cp.ins,
                                         reason="kv barrier after shard publish")
                        elif E_AG1:
                            if otp == H // 2 - 1:
                                all_gather(ag_kv_in[layer][:],
                                           ag_kv_cmb[rep][layer])
                        elif otp % 2 == 1:
                            g = otp // 2
                            all_gather(ag_kv_in[layer][g],
                                       ag_kv_out[rep][layer][g])

                    # ---- Q (rope) and U projections (local) ----
                    q_sb = acts.tile([128, H, T], bf16, name="q_sb", tag="q")
                    for otp in range(H // 2):
                        ps = proj_pair_psum(wofs + W_Q, otp, h)
                        for j in range(2):
                            ot = 2 * otp + j
                            qtmp = tmpp.tile([128, T], f32r, name="qtmp",
                                             tag="tmp")
                            nc.scalar.activation(
                                qtmp[:], ps[:, j, :], AF.Identity,
                                bias=bcol_sb[:, wofs + W_Q, ot:ot + 1])
                            rope_into(q_sb[:, ot, :], qtmp)

                    # stage per-iteration query halves (one dynamic DMA per
                    # w, all heads at once) so attention matmuls use static
                    # rhs addresses
                    qrep = acts.tile([128, NW, H, CH], bf16, name="qrep",
                                     tag="qrep")
                    for w in range(NW):
                        v_q = ((w % 2) * CH if (E_STATIC or E_SMM)
                               else sched_vals[w][2])
                        nc.sync.dma_start(qrep[:, w],
                                          q_sb[:, :, ds(v_q, CH)])

                    u_sb = acts.tile([128, H, T], bf16, name="u_sb", tag="u")
                    for otp in range(H // 2):
                        ps = proj_pair_psum(wofs + W_U, otp, h)
                        for j in range(2):
                            ot = 2 * otp + j
                            nc.scalar.activation(
                                u_sb[:, ot, :], ps[:, j, :], AF.Identity,
                                bias=bcol_sb[:, wofs + W_U, ot:ot + 1])

                    # ---- attention: 9 balanced causal block-pairs/head ----
                    # One dynamic 256KB slab fetch per (otp, w) covering both
                    # heads' K and V blocks; diagonal-first schedule so only
                    # w=0 and w=NW-1 need the data mask. The j heads share
                    # one psum tile and one silu per (otp, w).
                    a_sb = acts.tile([128, H, T], bf16, name="a_sb", tag="bigA")
                    if E_NOATTN:
                        nc.vector.tensor_copy(a_sb[:].rearrange(
                            "p h t -> p (h t)"), u_sb[:].rearrange(
                            "p h t -> p (h t)"))
                    for otp in range(0 if not E_NOATTN else H, H // 2):
                        ko = ag_kv_out[rep][layer][otp // 2]
                        pavs = [shrp.tile([128, 2, CH], f32, name=f"pav{j}",
                                          tag="shr",
                                          padded_shape=[128, 2, CH])
                                for j in range(2)]
                        for w in range(NW):
                            if E_STATIC:
                                v_c, v_s, v_h = w % 8, w % 2, w % 2
                            else:
                                v_c, v_s, _, v_h = sched_vals[w]
                            if E_SMM:
                                v_h = w % 2
                            kv = kfp.tile([128, 2, 2, CH], bf16, name="kv",
                                          tag="kf")
                            kvd = nc.sync.dma_start(
                                kv[:].rearrange("p (a b j) v q -> p a b j v q",
                                                a=1, b=1),
                                ko[ds(v_c, 1), ds(v_s, 1), otp % 2]
                                .rearrange("a b j v p q -> p a b j v q"))
                            if E_MANAG and not sim:
                                _add_dep(kvd.ins, kv_bar.ins,
                                         reason="kv fetch after barrier")
                            psc = pscp.tile([128, 2, 2, CH], f32, name="psc",
                                            tag="psc")
                            for j in range(2):
                                for kt in range(2):
                                    nc.tensor.matmul(
                                        psc[:, j, kt, :],
                                        kv[:, j, 0, kt * 128:(kt + 1) * 128],
                                        qrep[:, w, 2 * otp + j, :],
                                        start=True, stop=True)
                            wt = wtsp.tile([128, 2, 2, CH], bf16, name="wt",
                                           tag="wt")
                            nc.scalar.activation(wt[:], psc[:], AF.Silu,
                                                 scale=INV_SQRT_HD)
                            if w == 0 or w == NW - 1:
                                ms = 0 if w == 0 else 1
                                for j in range(2):
                                    nc.vector.tensor_mul(
                                        wt[:, j], wt[:, j],
                                        mask_sb[:, ms, :, :])
                            for j in range(2):
                                vft = kv[:, j, 1, :].rearrange(
                                    "p (k c) -> p k c", k=2)
                                for kt in range(2):
                                    if w == 0:
                                        dst = pavs[j][:, 0, :]
                                    elif w == NW - 1:
                                        dst = pavs[j][:, 1, :]
                                    else:
                                        dst = pavs[j][:, ds(v_h, 1), :]\
                                            .rearrange("p a b -> p (a b)")
                                    nc.tensor.matmul(
                                        dst, vft[:, kt, :], wt[:, j, kt, :],
                                        start=(w == 0 and kt == 0),
                                        stop=(w == NW - 1 and kt == 1))
                        for j in range(2):
                            nc.vector.tensor_mul(
                                a_sb[:, 2 * otp + j, :],
                                pavs[j][:].rearrange("p a b -> p (a b)"),
                                u_sb[:, 2 * otp + j, :])

                    # ---- output projection + residual ----
                    for otp in range(DT // 2):
                        ps = proj_pair_psum(wofs + W_O, otp, a_sb)
                        for j in range(2):
                            ot = 2 * otp + j
                            otmp = tmpp.tile([128, T], f32, name="otmp",
                                             tag="tmp")
                            nc.vector.tensor_scalar_add(
                                otmp[:], ps[:, j, :],
                                bcol_sb[:, wofs + W_O, ot:ot + 1])
                            nc.vector.tensor_add(x_sb[:, ot, :],
                                                 x_sb[:, ot, :], otmp[:])

                    # ---- FFN ----
                    h2 = layernorm(2 * layer + 1)
                    p_sb = acts.tile([128, DT, T], f32, name="p_sb", tag="p")
                    for otp in range(DT // 2):
                        ps = proj_pair_psum(wofs + W_1, otp, h2)
                        for j in range(2):
                            ot = 2 * otp + j
                            nc.scalar.activation(
                                p_sb[:, ot, :], ps[:, j, :], AF.Identity,
                                bias=bcol_sb[:, wofs + W_1, ot:ot + 1])
                    gp = acts.tile([128, DT, T], bf16, name="gp", tag="bigA")
                    for ot in range(DT):
                        sp = tmpp.tile([128, T], f32, name="sp", tag="tmp")
                        nc.scalar.activation(sp[:], p_sb[:, ot, :], AF.Silu)
                        nc.vector.tensor_mul(gp[:, ot, :], p_sb[:, ot, :], sp[:])
                    for otp in range(DT // 2):
                        ps = proj_pair_psum(wofs + W_2, otp, gp)
                        for j in range(2):
                            ot = 2 * otp + j
                            ftmp = tmpp.tile([128, T], f32, name="ftmp",
                                             tag="tmp")
                            nc.vector.tensor_scalar_add(
                                ftmp[:], ps[:, j, :],
                                bcol_sb[:, wofs + W_2, ot:ot + 1])
                            nc.vector.tensor_add(x_sb[:, ot, :],
                                                 x_sb[:, ot, :], ftmp[:])

                # ---- final layernorm + output ----
                hf = layernorm(2 * NL, affine=True)
                out_t_t = out_t.ap().rearrange("(dt p) t -> p dt t", p=128)
                nc.sync.dma_start(out_t_t[:], hf[:])

    nc.compile()
    return nc


# --------------------------------------------------------------------------
# host-side preparation
# --------------------------------------------------------------------------

def _tok_idx(c):
    r = c % 4
    return np.concatenate([np.arange(CH * r, CH * (r + 1)),
                           np.arange(CH * (7 - r), CH * (8 - r))])


def _host_prep(inputs):
    import ml_dtypes
    bf16 = ml_dtypes.bfloat16

    seqs = np.asarray(inputs["seqs"], np.float32)
    mask = np.asarray(inputs["attn_mask"])
    # The balanced-causal schedule only computes lower-triangular blocks and
    # skips the mask multiply on strictly-past blocks; both assume a causal
    # mask. Fail loudly if that ever changes.
    tril = np.tril(np.ones((L, L), dtype=bool))
    assert all(np.array_equal(mask[b], tril) for b in range(B)), \
        "kernel requires a causal (tril) attention mask"

    perm128 = np.concatenate([np.arange(0, 128, 2), np.arange(1, 128, 2)])
    perm_full = np.concatenate([h * 128 + perm128 for h in range(H)])

    def wprep(W):
        WT = np.ascontiguousarray(W.T)
        return np.ascontiguousarray(
            WT.reshape(DT, 128, 8, 128).transpose(2, 1, 0, 3))

    def bcolv(b):
        return np.ascontiguousarray(b.reshape(8, 128).T)

    def lncol(v):
        return np.ascontiguousarray(v.reshape(DT, 128).T)

    wstack, bstack = [], []
    for i in range(NL):
        ln1g = np.asarray(inputs["ln1_g"][i], np.float32)
        ln1b = np.asarray(inputs["ln1_b"][i], np.float32)
        ln2g = np.asarray(inputs["ln2_g"][i], np.float32)
        ln2b = np.asarray(inputs["ln2_b"][i], np.float32)
        for nm in ["Wq", "Wk", "Wu", "Wv", "Wo", "W1", "W2"]:
            Wm = np.asarray(inputs[nm][i], np.float32)
            bm = np.asarray(inputs["b" + nm[1:].lower()][i], np.float32)
            # fold the pre-projection layernorm affine into W and b
            if nm in ("Wq", "Wk", "Wu", "Wv"):
                bm = bm + Wm @ ln1b
                Wm = Wm * ln1g[None, :]
            elif nm == "W1":
                bm = bm + Wm @ ln2b
                Wm = Wm * ln2g[None, :]
            if nm in ("Wq", "Wk"):
                Wm = Wm[perm_full]
                bm = bm[perm_full]
            wstack.append(wprep(Wm))
            bstack.append(bcolv(bm))
    wstack = np.ascontiguousarray(np.stack(wstack)).astype(bf16)
    bstack = np.ascontiguousarray(np.stack(bstack), dtype=np.float32)

    lng = np.stack([lncol(np.asarray(inputs["ln1_g"][0], np.float32)),
                    lncol(np.asarray(inputs["ln2_g"][0], np.float32)),
                    lncol(np.asarray(inputs["ln1_g"][1], np.float32)),
                    lncol(np.asarray(inputs["ln2_g"][1], np.float32)),
                    lncol(np.asarray(inputs["lnf_g"], np.float32))])
    lnb = np.stack([lncol(np.asarray(inputs["ln1_b"][0], np.float32)),
                    lncol(np.asarray(inputs["ln2_b"][0], np.float32)),
                    lncol(np.asarray(inputs["ln1_b"][1], np.float32)),
                    lncol(np.asarray(inputs["ln2_b"][1], np.float32)),
                    lncol(np.asarray(inputs["lnf_b"], np.float32))])
    lng = np.ascontiguousarray(lng, dtype=np.float32)
    lnb = np.ascontiguousarray(lnb, dtype=np.float32)

    pos = np.arange(L, dtype=np.float32)
    ar = np.arange(0, HD, 2).astype(np.float32) / np.float32(HD)
    freqs = np.float32(1.0) / np.power(np.float32(10000.0), ar, dtype=np.float32)
    ang = pos[:, None] * freqs[None, :]
    sin_full, cos_full = np.sin(ang).astype(np.float32), np.cos(ang).astype(np.float32)

    pswap = np.zeros((128, 128), np.float32)
    for i in range(64):
        pswap[i, 64 + i] = 1.0
        pswap[64 + i, i] = 1.0

    in_maps = []
    for c in range(NC):
        b_idx, r = c // 4, c % 4
        tok = _tok_idx(c)
        cos_t = cos_full[tok].T
        sin_t = sin_full[tok].T
        sched = _sched_for_core(c)
        mblk = np.zeros((2, 2, 128, CH), np.float32)
        stab = np.zeros((NW, 4), np.int32)
        for w, (m, qh) in enumerate(sched):
            qc = r if qh == 0 else 7 - r
            if w == 0 or w == NW - 1:
                blk = mask[b_idx, CH * qc:CH * (qc + 1), CH * m:CH * (m + 1)]
                mblk[0 if w == 0 else 1] = blk.T.reshape(2, 128, CH)
            s = 1 if m >= 4 else 0
            r_own = m if m < 4 else 7 - m
            stab[w] = (b_idx * 4 + r_own, s, qh * CH, qh)
        in_maps.append({
            "x_fm": np.ascontiguousarray(seqs[b_idx, tok].T),
            "mblk": np.ascontiguousarray(mblk.astype(np.float32)).astype(bf16),
            "sched": np.ascontiguousarray(stab.reshape(1, NW * 4)),
            "cosf": np.ascontiguousarray(np.concatenate([cos_t, cos_t], 0)),
            "sinf": np.ascontiguousarray(np.concatenate([-sin_t, sin_t], 0)),
            "pswap": pswap, "onesf": np.ones((128, 128), np.float32),
            "wstack": wstack, "bstack": bstack, "lng": lng, "lnb": lnb,
        })
    return in_maps


def _get_program(unroll=1):
    key = ("nc", unroll)
    if key not in _CACHE:
        os.environ.setdefault("JAX_PLATFORMS", "")
        _CACHE[key] = _build_program(unroll=unroll)
    return _CACHE[key]


class _Runner:
    """Compile-once jitted SPMD runner over the axon/PJRT path."""

    def __init__(self, nc, donate=True):
        import jax
        from jax.experimental.shard_map import shard_map
        from jax.sharding import Mesh, PartitionSpec, NamedSharding
        import concourse.bass2jax as bass2jax
        import concourse.mybir as mybir

        self.jax = jax
        self.nc = nc
        bass2jax.install_neuronx_cc_hook()
        partition_name = (nc.partition_id_tensor.name
                          if nc.partition_id_tensor else None)
        in_names, out_names, out_avals, zero_outs = [], [], [], []
        for alloc in nc.m.functions[0].allocations:
            if not isinstance(alloc, mybir.MemoryLocationSet):
                continue
            name = alloc.memorylocations[0].name
            if alloc.kind == "ExternalInput":
                if name != partition_name:
                    in_names.append(name)
            elif alloc.kind == "ExternalOutput":
                out_names.append(name)
                shape = tuple(alloc.tensor_shape)
                dtype = mybir.dt.np(alloc.dtype)
                out_avals.append(jax.core.ShapedArray(shape, dtype))
                zero_outs.append(np.zeros(shape, dtype))
        self.in_names, self.out_names = in_names, out_names
        self.zero_outs = zero_outs
        n_params = len(in_names)
        all_names = in_names + out_names + (
            [partition_name] if partition_name else [])

        def _body(*args):
            operands = list(args)
            if partition_name is not None:
                operands.append(bass2jax.partition_id_tensor())
            return tuple(bass2jax._bass_exec_p.bind(
                *operands, out_avals=tuple(out_avals),
                in_names=tuple(all_names), out_names=tuple(out_names),
                lowering_input_output_aliases=(),
                sim_require_finite=True, sim_require_nnan=True, nc=nc))

        mesh = Mesh(np.asarray(jax.devices()[:NC]), ("core",))
        n_outs = len(out_names)
        self.fn = jax.jit(
            shard_map(_body, mesh=mesh,
                      in_specs=(PartitionSpec("core"),) * (n_params + n_outs),
                      out_specs=(PartitionSpec("core"),) * n_outs,
                      check_rep=False),
            donate_argnums=(tuple(range(n_params, n_params + n_outs))
                            if donate else ()),
            keep_unused=True)
        self.shard = NamedSharding(mesh, PartitionSpec("core"))

    def put_inputs(self, in_maps):
        jax = self.jax
        concat_in = [
            np.concatenate([np.asarray(in_maps[c][nm])[None]
                            for c in range(NC)], axis=0)
            .reshape(NC * in_maps[0][nm].shape[0], *in_maps[0][nm].shape[1:])
            for nm in self.in_names]
        return [jax.device_put(a, self.shard) for a in concat_in]

    def fresh_zeros(self):
        jax = self.jax
        return [jax.device_put(
            np.zeros((NC * z.shape[0], *z.shape[1:]), z.dtype), self.shard)
            for z in self.zero_outs]

    def run(self, in_arrs):
        outs = self.fn(*in_arrs, *self.fresh_zeros())
        self.jax.block_until_ready(outs)
        return outs

    def timed_k(self, in_arrs, zeros, k):
        """Wall time of k back-to-back async calls (single final block)."""
        jax = self.jax
        t0 = time.perf_counter()
        outs = None
        for _ in range(k):
            outs = self.fn(*in_arrs, *zeros)
        jax.block_until_ready(outs)
        return time.perf_counter() - t0


def _out_to_full(runner, outs):
    out = np.zeros((B, L, D), np.float32)
    arr0 = np.asarray(outs[runner.out_names.index("out_fm")]).reshape(NC, D, T)
    for c in range(NC):
        out[c // 4, _tok_idx(c)] = arr0[c].T
    return out


def kernel(**inputs):
    from concourse.bass_utils import run_bass_kernel_spmd
    in_maps = _host_prep(inputs)
    nc = _get_program(unroll=1)
    res = run_bass_kernel_spmd(nc, in_maps, core_ids=list(range(NC)))
    out = np.zeros((B, L, D), np.float32)
    for c in range(NC):
        out[c // 4, _tok_idx(c)] = res.results[c]["out_fm"].T
    return out


def bench_hw(inputs, unroll=12, unroll2=6, iters=6, k=9):
    """Correctness output + device-time estimate (slope method)."""
    if unroll2 >= unroll:
        unroll2 = max(1, unroll // 2)
    in_maps = _host_prep(inputs)
    r1 = _CACHE.setdefault("runner1", _Runner(_get_program(unroll=1),
                                              donate=False))
    rM = _CACHE.setdefault(f"runner{unroll2}",
                           _Runner(_get_program(unroll=unroll2),
                                   donate=False))
    rN = _CACHE.setdefault(f"runner{unroll}",
                           _Runner(_get_program(unroll=unroll),
                                   donate=False))
    in1 = r1.put_inputs(in_maps)
    inM = rM.put_inputs(in_maps)
    inN = rN.put_inputs(in_maps)
    z1 = r1.fresh_zeros()
    zM = rM.fresh_zeros()
    zN = rN.fresh_zeros()
    import jax
    for z in (z1, zM, zN):
        jax.block_until_ready(z)
    outs = r1.fn(*in1, *z1)
    jax.block_until_ready(outs)
    full = _out_to_full(r1, outs)
    fullN = _out_to_full(rN, rN.fn(*inN, *zN))
    assert np.allclose(full, fullN, atol=3e-5), "unrolled output mismatch"
    rM.timed_k(inM, zM, 2)
    rN.timed_k(inN, zN, 2)
    k0 = max(2, k // 3)
    raw = []
    for _ in range(iters):
        tM0 = rM.timed_k(inM, zM, k0)
        tMk = rM.timed_k(inM, zM, k)
        tN0 = rN.timed_k(inN, zN, k0)
        tNk = rN.timed_k(inN, zN, k)
        raw.append((tM0, tMk, tN0, tNk))
    ests = [((r[3] - r[2]) - (r[1] - r[0])) / (k - k0) / (unroll - unroll2)
            for r in raw]
    mins = [min(r[i] for r in raw) for i in range(4)]
    est = float(np.median(ests))
    if est <= 0:
        sM = (mins[1] - mins[0]) / (k - k0)
        sN = (mins[3] - mins[2]) / (k - k0)
        est = max((sN - sM) / (unroll - unroll2), 1e-9)
    return full, est, {"t1": mins[0], "tN": mins[2], "unroll": unroll,
                       "t1s": [e * 1e3 for e in ests],
                       "tNs": [r[3] for r in raw]}
